# revision 2
# baseline (speedup 1.0000x reference)
"""Cost-volume kernel for Trainium2 (Bass/Tile), SPMD over 8 NeuronCores.

out[n, c, d, h, x] = l[n, c, h, x] - r[n, c, h, x - d]  for x >= d, else 1.0
shapes: l, r = (2, 32, 128, 256) f32 -> out = (2, 32, 48, 128, 256) f32

Sharding: the 64 (n, c) pairs split 8 ways -> G=8 channels per core; no
cross-core communication. Output-write bound (~50 MB/core at ~358 GB/s HBM).

Per-core layout: SBUF partition p = (g, h_hi) with per-partition free dims
(h_lo=8, w=256) so every DMA descriptor covers an 8 KB contiguous DRAM line.
DRAM tensors are plain-contiguous; element order (g, h_hi, h_lo, w) matches
partition-major order, so no AP rearranges are needed. One DVE subtract per
disparity d (all channels at once), ones-fill via GpSimd memset, one 1 MiB
output DMA per d alternating across the two HWDGE rings.
"""

import numpy as np

import concourse.bacc as bacc
import concourse.mybir as mybir
import concourse.tile as tile
from concourse.bass_utils import run_bass_kernel_spmd

MAX_DISP = 48
N, C, H, W = 2, 32, 128, 256
NCORES = 8
G = (N * C) // NCORES  # 8 (n, c) channels per core
HL = 8  # h_lo rows packed per partition; 128 partitions = G * (H // HL)

_CACHE = {}


def build_bass():
    if "nc" in _CACHE:
        return _CACHE["nc"]
    nc = bacc.Bacc("TRN2", target_bir_lowering=False, debug=False)
    l = nc.dram_tensor("l", (G, H, W), mybir.dt.float32, kind="ExternalInput")
    r = nc.dram_tensor("r", (G, H, W), mybir.dt.float32, kind="ExternalInput")
    out = nc.dram_tensor(
        "out", (G, MAX_DISP, H, W), mybir.dt.float32, kind="ExternalOutput"
    )

    with tile.TileContext(nc) as tc:
        with tc.tile_pool(name="inp", bufs=1) as inpool, tc.tile_pool(
            name="outp", bufs=8
        ) as outpool:
            l_sb = inpool.tile([128, HL, W], mybir.dt.float32)
            r_sb = inpool.tile([128, HL, W], mybir.dt.float32)
            nc.sync.dma_start(out=l_sb[:], in_=l.ap())
            nc.scalar.dma_start(out=r_sb[:], in_=r.ap())
            for d in range(MAX_DISP):
                t = outpool.tile([128, HL, W], mybir.dt.float32)
                if d > 0:
                    nc.gpsimd.memset(t[:, :, :d], 1.0)
                nc.vector.tensor_sub(t[:, :, d:], l_sb[:, :, d:], r_sb[:, :, : W - d])
                eng = nc.sync if d % 2 == 0 else nc.scalar
                eng.dma_start(out=out.ap()[:, d, :, :], in_=t[:])

    nc.compile()
    _CACHE["nc"] = nc
    return nc


def make_in_maps(l_fmap, r_fmap):
    l_flat = np.ascontiguousarray(l_fmap, dtype=np.float32).reshape(N * C, H, W)
    r_flat = np.ascontiguousarray(r_fmap, dtype=np.float32).reshape(N * C, H, W)
    return [
        {
            "l": np.ascontiguousarray(l_flat[k * G : (k + 1) * G]),
            "r": np.ascontiguousarray(r_flat[k * G : (k + 1) * G]),
        }
        for k in range(NCORES)
    ]


def gather(results):
    out = np.concatenate([res["out"][None] for res in results], axis=0)
    return out.reshape(N, C, MAX_DISP, H, W)


def kernel(l_fmap, r_fmap):
    nc = build_bass()
    in_maps = make_in_maps(l_fmap, r_fmap)
    res = run_bass_kernel_spmd(nc, in_maps, core_ids=list(range(NCORES)))
    return gather(res.results)
